# revision 13
# baseline (speedup 1.0000x reference)
"""Trainium2 Bass kernel for M2 Cartesian group-equivariant conv.

Problem: x [8,32,8,128,128] f32, weights [32,32,125] f32 ->
out [8,32,8,124,124] f32, where the conv kernel stack
ks[Cout,Cin,Oout,5,5,5] is a B-spline expansion of `weights` and the
orientation axis is convolved with wrap padding.

Strategy:
  - Data parallel over batch: core b processes image b.
  - Fold orientation into channels: 256 = 8*32 input channels,
    256 = 32*8 output channels. For each of the 25 spatial taps
    (dy,dx) the orientation part of the conv becomes a dense 256x256
    block matrix W2 (banded-circulant over orientations; zeros padded
    in - 1.6x FLOPs for full 128x128 PE utilization).
  - Per output tile of 4 rows x 124 cols (=496 PSUM columns), one core
    accumulates 2 kblocks x 25 taps = 50 bf16 matmuls
    [128x128]^T @ [128x496] into one PSUM bank.
  - bf16 operands, fp32 PSUM accumulate. Measured norm rel err vs the
    fp32 reference: ~2.4e-3.
"""

import numpy as np
import ml_dtypes
from math import pi

try:
    import concourse.bass as bass
except ImportError:  # stand-alone grading dir: concourse lives in the repo
    import sys

    sys.path.insert(0, "/opt/trn_rl_repo")
    import concourse.bass as bass

import concourse.mybir as mybir
import concourse.tile as tile
from concourse import bacc
from concourse.bass_utils import run_bass_kernel_spmd

BF16 = ml_dtypes.bfloat16

B = 8
CIN = 32
COUT = 32
ORIENT = 8
K = 5
H = W = 128
HO = WO = H - K + 1  # 124
NCH = ORIENT * CIN  # 256
COLS_IN = H * W  # 16384
NT = HO // 4  # 31 output tiles of 4 rows each
NFREE = 4 * WO  # 496 psum columns per tile

# Set by test.py to collect a profile; harness leaves it off.
TRACE = False
TRACE_DIR = None
LAST_RESULT = None

_PROGRAM = None


def _kernel_stack(weights):
    """numpy mirror of reference._kernel_stack -> [Cout,Cin,O,K,K,K] f32."""

    def b2(t):
        a = np.abs(t)
        return np.where(a <= 0.5, 0.75 - a * a,
                        np.where(a < 1.5, 0.5 * (1.5 - a) ** 2, 0.0))

    off = (K - 1) / 2.0
    xs = np.linspace(-off, off, K)
    og, yg, xg = np.meshgrid(xs, xs, xs, indexing="ij")
    nodes = np.stack([og, yg, xg], axis=-1).reshape(-1, 3)
    angles = -np.arange(ORIENT, dtype=np.float64) * (2.0 * pi / ORIENT)
    c = np.cos(angles)[:, None, None, None]
    s = np.sin(angles)[:, None, None, None]
    xr = xg * c + yg * s
    yr = -xg * s + yg * c
    grid = np.stack([np.broadcast_to(og, xr.shape), yr, xr], axis=-1)
    basis = b2(grid[..., None, :] - nodes).prod(axis=-1)  # [O,K,K,K,125]
    return np.einsum("uvn,oabcn->uvoabc", weights.astype(np.float64),
                     basis).astype(np.float32)


def _build_w_lhst(weights):
    """[128 p_k, 25 tap, 2 kb, 2 nb, 128 p_n] bf16 stationary operands.

    n = nb*128 + p_n encodes (co, i) as co*8 + i.
    k = kb*128 + p_k encodes (o, ci) as o*32 + ci.
    """
    ks = _kernel_stack(weights)  # [Cout,Cin,O,do,dy,dx]
    W2 = np.zeros((256, 256, K, K), np.float32)  # [n, k, dy, dx]
    for n in range(256):
        co, i = divmod(n, 8)
        for o in range(ORIENT):
            do = (o - i + 2) % ORIENT
            if do < K:
                W2[n, o * 32:(o + 1) * 32] = ks[co, :, i, do]
    w = W2.reshape(2, 128, 2, 128, K * K)  # [nb,p_n,kb,p_k,tap]
    w = w.transpose(3, 4, 2, 0, 1)  # [p_k,tap,kb,nb,p_n]
    return np.ascontiguousarray(w).astype(BF16)


def _build_program():
    nc = bacc.Bacc()
    xin = nc.declare_dram_parameter("xin", [2, 128, COLS_IN],
                                    mybir.dt.bfloat16, isOutput=False)
    wts = nc.declare_dram_parameter("wts", [128, K * K, 2, 2, 128],
                                    mybir.dt.bfloat16, isOutput=False)
    out = nc.declare_dram_parameter("out", [2, 128, HO * WO],
                                    mybir.dt.float32, isOutput=True)

    with tile.TileContext(nc) as tc:
        with (
            tc.tile_pool(name="xp", bufs=1) as xpool,
            tc.tile_pool(name="wp", bufs=1) as wpool,
            tc.tile_pool(name="ps", bufs=8, space="PSUM") as pspool,
            tc.tile_pool(name="op", bufs=4) as opool,
        ):
            x_sb = xpool.tile([128, 2, COLS_IN], mybir.dt.bfloat16)
            w_sb = wpool.tile([128, K * K, 2, 2, 128], mybir.dt.bfloat16)

            # ~3.4us of throwaway matmuls: releases the HAM clock gate
            # (cold 1.2GHz -> warm 2.4GHz) while the input DMA is still
            # in flight, so the real matmuls start at full clock
            warm_ps = pspool.tile([128, NFREE], mybir.dt.float32, tag="ps")
            # chunked input DMA (col-major, both kblocks per chunk) so the
            # first banks' matmuls start after ~1 chunk instead of the
            # whole 8.4MB load; weights split per-tap and interleaved after
            # the first chunks for the same reason
            NCHUNK = 16
            csz = COLS_IN // NCHUNK

            def x_chunk(cchunk):
                for q in range(2):
                    nc.sync.dma_start(
                        out=x_sb[:, q, cchunk * csz:(cchunk + 1) * csz],
                        in_=xin[q, :, cchunk * csz:(cchunk + 1) * csz])

            # interleave so the PE can start after (w tap0-2, x chunk0-1)
            # and never starves: weight taps arrive faster than the first
            # bank consumes them
            nc.sync.dma_start(out=w_sb[:, 0:3], in_=wts[:, 0:3])
            # ~3.4us of throwaway matmuls on the first weight chunk:
            # releases the HAM clock gate (cold 1.2GHz -> warm 2.4GHz)
            # while the input DMA is still in flight, so the real matmuls
            # start at full clock with the PE never idle
            warm_lhs = w_sb[:, 0, 0, 0, :]
            for _ in range(32):
                nc.tensor.matmul(warm_ps[:, 0:128], lhsT=warm_lhs,
                                 rhs=warm_lhs, start=True, stop=True)
            x_chunk(0)
            x_chunk(1)
            for tap in range(3, K * K, 3):
                hi = min(tap + 3, K * K)
                nc.sync.dma_start(out=w_sb[:, tap:hi], in_=wts[:, tap:hi])
            for cchunk in range(2, NCHUNK):
                x_chunk(cchunk)

            # row-major view of the image for strided sliding windows
            xrows = [x_sb[:, q, :].rearrange("p (r c) -> p r c", c=W)
                     for q in range(2)]

            for t in range(NT):
                for nb in range(2):
                    ps = pspool.tile([128, NFREE], mybir.dt.float32)
                    idx = 0
                    for kb in range(2):
                        for tap in range(K * K):
                            dy, dx = divmod(tap, K)
                            rhs = xrows[kb][:, 4 * t + dy:4 * t + dy + 4,
                                            dx:dx + WO]
                            nc.tensor.matmul(
                                ps,
                                lhsT=w_sb[:, tap, kb, nb, :],
                                rhs=rhs,
                                start=(idx == 0),
                                stop=(idx == 49),
                            )
                            idx += 1
                    ot = opool.tile([128, NFREE], mybir.dt.float32)
                    nc.vector.tensor_copy(ot, ps)
                    nc.sync.dma_start(
                        out=out[nb, :, t * NFREE:(t + 1) * NFREE], in_=ot)
    nc.compile()
    return nc


def kernel(x, weights):
    global _PROGRAM, LAST_RESULT
    x = np.asarray(x, dtype=np.float32)
    weights = np.asarray(weights, dtype=np.float32)
    assert x.shape == (B, CIN, ORIENT, H, W)

    if _PROGRAM is None:
        _PROGRAM = _build_program()
    nc = _PROGRAM

    w_lhst = _build_w_lhst(weights)
    in_maps = []
    for b in range(B):
        xc = x[b].transpose(1, 0, 2, 3).reshape(NCH, COLS_IN)  # ch = o*32+ci
        xc = xc.reshape(2, 128, COLS_IN).astype(BF16)
        in_maps.append({"xin": xc, "wts": w_lhst})

    kwargs = {}
    if TRACE:
        kwargs.update(trace=True, tmpdir=TRACE_DIR)
    res = run_bass_kernel_spmd(nc, in_maps, list(range(B)), **kwargs)
    LAST_RESULT = res

    full = np.empty((B, COUT, ORIENT, HO, WO), np.float32)
    for b in range(B):
        o = np.asarray(res.results[b]["out"])  # [2,128,15376]
        full[b] = o.reshape(256, HO, WO).reshape(COUT, ORIENT, HO, WO)
    return full
